# revision 4
# baseline (speedup 1.0000x reference)
"""Trainium2 Bass kernel for nn_DepthEstimationNet (vq_codebook).

reference:  d = x.reshape(B, S);  ratio[b,i,j] = d[b,i] * (1/d[b,j])
            out[b,i,j] = inv[searchsorted(q, ratio, side='right')]
shapes:     x [8,1,48,48] -> out [8, 2304, 2304] fp32 (~170 MB)

Strategy (data-parallel over batch, one batch per NeuronCore):
  out = inv[0] + sum_{k=1..40} (inv[k]-inv[k-1]) * [v >= q[k-1]]
  (thresholds sorted -> indicator sequence is nested, so the telescoped
  sum reproduces inv[idx] exactly up to fp32 summation noise ~1e-6 rel).

  - host computes recip = fp32(1/d) per batch (bit-identical rounding to
    the reference) and replicates it across 128 SBUF partitions.
  - per 128-row tile: v = d_col * recip (same fp32 rounding as the
    reference), then 20 single-source custom DVE ops, each evaluating
    TWO terms:  p_m = (v>=q_a)*c_a + (v>=q_b)*c_b
    (q_a, c_a, q_b as instruction scalars; c_b per-partition via in1/C3).
    Compares are exact fp32 -> binning is exact.
  - the 20 partials are accumulated by the PE array (identity-weight
    matmuls into PSUM, fp32): runs concurrently with the DVE.
  - ACT evacuates PSUM -> SBUF adding inv[0]; sync DMAs tiles out.
  - q/inv enter as instruction immediates (same for all cores -> SPMD).
"""
import numpy as np

S = 2304          # 48*48
P = 128           # partitions
NT = S // P       # 18 row tiles per batch
NB = 40           # thresholds
B = 8             # batch == cores
NOP = NB // 2     # 20 custom DVE ops (2 terms each)
CHUNKS = (512, 512, 512, 512, 256)   # PSUM bank-sized column chunks

_CACHE = {}


def _register_ops():
    import dataclasses
    import concourse.dve_ops as dve_ops_mod
    from concourse.dve_spec import Spec, Src0, C0, C1, C2, C3, _spill_c3_to_src1
    from concourse.dve_ops import DveOp, OPS
    from concourse.dve_table_gen import dve_ver_for

    def reg(name, spec):
        for op in OPS:
            if op.name == name:
                return op
        op = DveOp(name, spec, subdim=False, uops_sha={})
        OPS.append(op)
        dve_ops_mod._SUB_OPCODE_FOR_NAME[name] = (
            dve_ops_mod._CUSTOM_DVE_ROW_BASE + len(OPS) - 1
        )
        assert dve_ops_mod._SUB_OPCODE_FOR_NAME[name] < 0x20
        dve_ops_mod.CUSTOM_DVE_SPECS[name] = spec
        ver = dve_ver_for("TRN2")
        try:
            op.compile(ver)
            return op
        except ValueError as e:
            import re
            m = re.search(r'uops_sha\["' + ver + r'"\]="([0-9a-f]+)"', str(e))
            assert m, f"no sha in: {e}"
            op2 = dataclasses.replace(op, uops_sha={ver: m.group(1)})
            OPS[OPS.index(op)] = op2
            return op2

    # p = (v >= q_a)*c_a + (v >= q_b)*c_b ; c_b rides in1 (C3 spill)
    body = _spill_c3_to_src1((Src0 >= C0) * C1 + (Src0 >= C2) * C3)
    spec = Spec(
        body=body,
        reference=lambda in0, in1, s0, s1, imm2: (in0 >= s0) * s1
        + (in0 >= imm2) * in1,
    )
    return reg("ANT_MASKPAIR", spec)


def _build_nc(q, inv, repeat=1, tiny_out=False):
    import concourse.bass as bass
    import concourse.mybir as mybir

    MASKPAIR = _register_ops()
    f32 = mybir.dt.float32

    inv64 = inv.astype(np.float64)
    dinv = (inv64[1:] - inv64[:-1]).astype(np.float32)   # [40]
    inv0 = float(inv[0])

    nc = bass.Bass()
    r_in = nc.declare_dram_parameter("recipb", [P, S], f32, isOutput=False)
    d_in = nc.declare_dram_parameter("dcol", [P, NT], f32, isOutput=False)
    i_in = nc.declare_dram_parameter("ident", [P, P], f32, isOutput=False)
    c_in = nc.declare_dram_parameter("cb", [P, NOP + 1], f32, isOutput=False)
    out_shape = [P, 8] if tiny_out else [S, S]
    y_out = nc.declare_dram_parameter("out", out_shape, f32, isOutput=True)
    y_big = (
        nc.dram_tensor("scratch", [S, S], f32, kind="Internal")
        if tiny_out
        else y_out
    )

    # per-iteration semaphore totals
    T_PP = NT * NOP          # partials produced / consumed
    T_AE = NT                # tiles evacuated by ACT
    T_OD = 16 * NT           # output DMAs

    with (
        nc.sbuf_tensor("rb", [P, S], f32) as rb,
        nc.sbuf_tensor("dc", [P, NT], f32) as dc,
        nc.sbuf_tensor("idw", [P, P], f32) as idw,
        nc.sbuf_tensor("cbb", [P, NOP + 1], f32) as cbb,
        nc.sbuf_tensor("v", [P, S], f32) as v,
        nc.sbuf_tensor("p0", [P, S], f32) as p0,
        nc.sbuf_tensor("p1", [P, S], f32) as p1,
        nc.sbuf_tensor("o0", [P, S], f32) as o0,
        nc.sbuf_tensor("o1", [P, S], f32) as o1,
        nc.psum_tensor("acc", [P, S], f32) as acc,
        nc.Block() as block,
        nc.semaphore("in_sem") as in_sem,
        nc.semaphore("pprod") as pprod,
        nc.semaphore("pcons") as pcons,
        nc.semaphore("aevac") as aevac,
        nc.semaphore("out_dma") as out_dma,
    ):
        pb = (p0, p1)
        ob = (o0, o1)
        col0 = [0, 512, 1024, 1536, 2048]

        @block.sync
        def _(sync):
            sync.dma_start(out=rb[:], in_=r_in[:]).then_inc(in_sem, 16)
            sync.dma_start(out=dc[:], in_=d_in[:]).then_inc(in_sem, 16)
            sync.dma_start(out=idw[:], in_=i_in[:]).then_inc(in_sem, 16)
            sync.dma_start(out=cbb[:], in_=c_in[:]).then_inc(in_sem, 16)
            for r in range(repeat):
                for t in range(NT):
                    sync.wait_ge(aevac, r * T_AE + t + 1)
                    sync.dma_start(
                        out=y_big[t * P:(t + 1) * P, :], in_=ob[t % 2][:]
                    ).then_inc(out_dma, 16)
            sync.wait_ge(out_dma, repeat * T_OD)
            if tiny_out:
                sync.dma_start(out=y_out[:], in_=o1[:, 0:8]).then_inc(in_sem, 16)
                sync.wait_ge(in_sem, 80)

        @block.vector
        def _(vector):
            vector.wait_ge(in_sem, 64)
            for r in range(repeat):
                for t in range(NT):
                    vector.tensor_scalar_mul(v[:], rb[:], dc[:, t:t + 1])
                    for m in range(NOP):
                        g = r * T_PP + t * NOP + m
                        if g >= 2:
                            vector.wait_ge(pcons, g - 1)
                        vector._custom_dve(
                            MASKPAIR,
                            out=pb[g % 2][:],
                            in0=v[:],
                            in1=cbb[:, m:m + 1],
                            s0=float(q[2 * m]),
                            s1=float(dinv[2 * m]),
                            imm2=float(q[2 * m + 1]),
                        ).then_inc(pprod, 1)

        @block.tensor
        def _(tensor):
            for r in range(repeat):
                for t in range(NT):
                    if r * T_AE + t >= 1:
                        tensor.wait_ge(aevac, r * T_AE + t)
                    for m in range(NOP):
                        g = r * T_PP + t * NOP + m
                        tensor.wait_ge(pprod, g + 1)
                        src = pb[g % 2]
                        for ci, (c0, w) in enumerate(zip(col0, CHUNKS)):
                            ins = tensor.matmul(
                                acc[:, c0:c0 + w],
                                idw[:],
                                src[:, c0:c0 + w],
                                start=(m == 0),
                                stop=(m == NOP - 1),
                                skip_group_check=True,
                            )
                        ins.then_inc(pcons, 1)

        @block.scalar
        def _(scalar):
            for r in range(repeat):
                for t in range(NT):
                    g1 = r * T_PP + (t + 1) * NOP
                    scalar.wait_ge(pcons, g1)
                    k = r * T_AE + t
                    if k >= 2:
                        scalar.wait_ge(out_dma, 16 * (k - 1))
                    import concourse.mybir as mybir
                    for c0, w in zip(col0, CHUNKS):
                        ins = scalar.activation(
                            ob[t % 2][:, c0:c0 + w], acc[:, c0:c0 + w],
                            mybir.ActivationFunctionType.Identity,
                            bias=cbb[:, NOP:NOP + 1], scale=1.0,
                        )
                    ins.then_inc(aevac, 1)

    from concourse.library_overlay import lower_extended_insts
    lower_extended_insts(nc)
    return nc


def _in_maps(x, q, inv):
    d = x.reshape(B, S).astype(np.float32)
    recip = (np.float32(1.0) / d).astype(np.float32)
    inv64 = inv.astype(np.float64)
    dinv = (inv64[1:] - inv64[:-1]).astype(np.float32)
    ident = np.eye(P, dtype=np.float32)
    cvals = np.concatenate([dinv[1::2], np.float32(inv[0]).reshape(1)])
    cb = np.ascontiguousarray(
        np.broadcast_to(cvals[None, :], (P, NOP + 1)).astype(np.float32)
    )
    maps = []
    for b in range(B):
        maps.append({
            "recipb": np.ascontiguousarray(np.broadcast_to(recip[b], (P, S))),
            "dcol": np.ascontiguousarray(d[b].reshape(NT, P).T),
            "ident": ident,
            "cb": cb,
        })
    return maps


def kernel(x, q, inv):
    x = np.asarray(x, dtype=np.float32)
    q = np.asarray(q, dtype=np.float32)
    inv = np.asarray(inv, dtype=np.float32)
    assert x.shape == (B, 1, 48, 48)

    key = (q.tobytes(), inv.tobytes())
    if key not in _CACHE:
        _CACHE[key] = _build_nc(q, inv)
    nc = _CACHE[key]

    from concourse.bass_utils import run_bass_kernel_spmd
    res = run_bass_kernel_spmd(nc, _in_maps(x, q, inv), list(range(B)))
    out = np.stack([res.results[b]["out"] for b in range(B)], axis=0)
    return out
